# revision 75
# baseline (speedup 1.0000x reference)
"""Multi-head attention Trainium2 Bass kernel, sharded over 8 NeuronCores.

Problem: B=4, S=2048, D=1024, H=16 heads (DK=64), fp32, random 0/1 mask.

Sharding (data-parallel batch x tensor-parallel heads):
  core c handles batch b = c // 2, head-group hg = c % 2 (8 heads = 512 dims).
  Each core computes Q/K/V projections for its head-group, masked softmax
  attention for its 8 heads, and a partial output projection over its 512
  contraction dims. The host sums the two partials per batch (the "all-reduce"
  is a host-side pairwise add since we gather outputs anyway).

On-device layout (per core):
  x inputs, K^T, Q^T, V, mask and all weights live in bf16 (the BIR verifier
  forbids mixing f32/f32r with 16-bit matmul operands). K^T is resident
  [512, 2048] (head dim on partitions); Q^T is computed per 512-wide
  q-chunk. Scores come out transposed, S^T = [k, q], with the two heads of
  a pair row-packed into disjoint halves of the PE array; exp() runs on the
  scalar engine out of PSUM (2 banks per ACTIVATE), EXCEPT one k-tile in
  four whose exp is offloaded to the vector+gpsimd pair as a Schraudolph
  bit-trick fast exp (int32(A*x+B) convert on the DVE -- gpsimd cannot read
  PSUM -- then a bitcast multiply with the mask on gpsimd; those units'
  scores use the psF banks so the sps rotation stays dedicated to the
  scalar-engine pipeline). The remaining mask multiplies are split between
  gpsimd and the vector engine (bf16 2x mode).

  P@V runs TRANSPOSED: the masked-exp tile E[k, q] is the STATIONARY operand
  (128 k x 128 q blocks) and [ones | V_h] (65 cols) is the moving operand, so
  each matmul produces C'[q, (z, d)] = 128 q-rows x 65 cols in 65 cycles --
  full PE-array occupancy (the [d, q] orientation would waste half the array
  on 65-row outputs). The softmax sum over k lands as COLUMN 0 of each
  accumulator, i.e. a per-partition scalar: reciprocal + tensor_scalar
  normalize it without any partition broadcast or shift. Four 65-col
  accumulator groups share each PSUM bank (only the first group issues
  start=True; the bank-wide pending-zero makes the siblings' first write a
  fresh write). The normalized C'[q, d] bf16 tiles are transposed back to
  [d, q] by the PE (against a persistent identity) straight into one PSUM
  bank laid out as cpair[(hh, d), (qsub, q)], which one tensor_copy moves to
  SBUF for the output projection.

  PSUM discipline (8 banks): psS = 2 x [128,2,512] score tiles (4 banks),
  psC = 2 x [128,512] C' accumulator banks (pair j+1 overlaps pair j's
  normalize thanks to the boundary filler lumps), psF = 2 x [128,512] filler
  banks hosting the K / Q / V projection groups, output-projection groups,
  and the transpose targets.

  Schedule: head = [DMA, K group (0,0), Q group (0,0)]; the rest of the K/Q
  projections spread dt-major across qc0's head pairs and the V projection
  rides inside the kt loop of (qc0, j0), all just in time for the k tiles
  that consume them (the head and qc0 are DMA-bandwidth-bound, so V/P@V are
  additionally deferred several k tiles behind the scores there). The
  output projection of chunk qc and the Q projection of chunk qc+1 are
  emitted as one-group filler closures popped between kt units. Pair
  finishes (trailing P@V + normalize) are deferred into the next pair's
  first unit and the transposes into its filler slots, so head-pair
  boundaries never idle the scalar engine; pair j3 finishes inline because
  its transpose gates the next chunk's output projection.

  Output is produced transposed ([1024, 2048] per core); host re-transposes
  and sums the two head-group partials per batch.
"""
import numpy as np

import concourse.bass as bass
import concourse.mybir as mybir
import concourse.tile as tile
from concourse import bacc
from concourse import masks

B, S, D, H = 4, 2048, 1024, 16
DK = D // H          # 64
NCORES = 8
HG = 2               # head groups (tensor-parallel factor per batch)
HPG = H // HG        # 8 heads per core
DH = D // HG         # 512 head dims per core
QCN = 4              # q chunks
QCS = S // QCN       # 512
KT = S // 128        # 16 k tiles
DT = D // 128        # 8 contraction tiles for projections
F32 = mybir.dt.float32
F32R = mybir.dt.float32r
BF16 = mybir.dt.bfloat16

# Schraudolph fast-exp constants: exp(x) ~ bitcast_f32(int32(A*x + B))
FEXP_A = float(2.0 ** 23 / np.log(2.0))
FEXP_B = float(127.0 * 2 ** 23 - 486411.0)


def build_nc():
    nc = bacc.Bacc(None)
    xqT = nc.declare_dram_parameter("xqT", [D, S], BF16, isOutput=False)
    xkT = nc.declare_dram_parameter("xkT", [D, S], BF16, isOutput=False)
    xvT = nc.declare_dram_parameter("xvT", [D, S], BF16, isOutput=False)
    maskT = nc.declare_dram_parameter("maskT", [S, S], BF16, isOutput=False)
    wqT = nc.declare_dram_parameter("wqT", [D, DH], BF16, isOutput=False)
    wkT = nc.declare_dram_parameter("wkT", [D, DH], BF16, isOutput=False)
    wvT = nc.declare_dram_parameter("wvT", [D, DH], BF16, isOutput=False)
    woT = nc.declare_dram_parameter("woT", [DH, D], BF16, isOutput=False)
    bq2 = nc.declare_dram_parameter("bq2", [128, DH // 128], F32, isOutput=False)
    bk2 = nc.declare_dram_parameter("bk2", [128, DH // 128], F32, isOutput=False)
    vr2 = nc.declare_dram_parameter("vr2", [128, D // 128], F32, isOutput=False)
    outT = nc.declare_dram_parameter("outT", [D, S], F32, isOutput=True)

    with tile.TileContext(nc) as tc:
        with (
            tc.tile_pool(name="persist", bufs=1) as pp,
            tc.tile_pool(name="w_a", bufs=1) as wpool,
            tc.tile_pool(name="x_a", bufs=2) as xpool,
            tc.tile_pool(name="work", bufs=2) as wp,
            tc.tile_pool(name="psS", bufs=2, space="PSUM") as psS,
            tc.tile_pool(name="psC", bufs=2, space="PSUM") as psC,
            tc.tile_pool(name="psF", bufs=2, space="PSUM") as psF,
        ):
            # ---- persistent tiles ----
            wo_full = pp.tile([128, DH // 128, D], BF16, tag="wo", name="wo_full")
            wo_sb = [wo_full[:, i, :] for i in range(DH // 128)]
            bias_sb = pp.tile([128, 2 * (DH // 128) + D // 128], F32, tag="bias",
                              name="bias_sb")
            bq_sb = bias_sb[:, 0:DH // 128]
            bk_sb = bias_sb[:, DH // 128:2 * (DH // 128)]
            vr_sb = bias_sb[:, 2 * (DH // 128):]
            wq_full = pp.tile([128, DT, DH], BF16, tag="wq", name="wq_full")
            wq_sb = [wq_full[:, i, :] for i in range(DT)]
            kt_sb = [pp.tile([128, S], BF16, tag=f"kt{i}", name=f"kt{i}")
                     for i in range(DH // 128)]
            v_full = pp.tile([128, KT, HPG * 65], BF16, tag="v", name="v_full")
            v_sb = [v_full[:, i, :] for i in range(KT)]
            ident = pp.tile([128, 128], BF16, tag="ident", name="ident")
            masks.make_identity(nc, ident[:])

            wk_full = wpool.tile([128, DT, DH], BF16, tag="wk", name="wk_full")
            wv_full = wpool.tile([128, DT, DH], BF16, tag="wv", name="wv_full")
            wk_sb = [wk_full[:, i, :] for i in range(DT)]
            wv_sb = [wv_full[:, i, :] for i in range(DT)]

            def sps_tile(name):
                return psS.tile([128, 2, QCS], F32, tag="sps", name=name, bufs=2)

            def cacc_tile(name):
                return psC.tile([128, QCS], F32, tag="cps", name=name, bufs=2)

            def fil_tile(name):
                return psF.tile([128, QCS], F32, tag="fil", name=name, bufs=2)

            # DMA issue order ~= transfer order (global DMA device). The
            # exp-start critical path is wk/xk0 (K group 0) then wq/xq0
            # (Q group 0) then the first mask piece; everything else is
            # issued later (or sits behind a blocking instruction in its
            # engine stream) so it cannot delay these.
            nc.gpsimd.dma_start(bq_sb[:, :], bq2[:])
            nc.gpsimd.dma_start(bk_sb[:, :], bk2[:])
            nc.gpsimd.dma_start(vr_sb[:, :], vr2[:])
            for i in range(KT):
                ones_cols = v_sb[i].rearrange("p (h c) -> p h c", h=HPG)[:, :, 0:1]
                nc.gpsimd.memset(ones_cols, 1.0)

            # xk chunk tiles (4 live: chunks 1-3 are consumed inside qc0)
            xk_tiles = {}

            def load_xk(kc, eng):
                x_t = xpool.tile([128, DT, QCS], BF16, tag="x", name="x_k", bufs=4)
                xs = xkT[:, kc * QCS:(kc + 1) * QCS].rearrange(
                    "(t p) s -> p t s", p=128)
                eng.dma_start(x_t[:], xs)
                xk_tiles[kc] = x_t

            def load_xk0_interleaved():
                """wk[i] / xk0[i] slice pairs so K group (0,0) starts early."""
                x_t = xpool.tile([128, DT, QCS], BF16, tag="x", name="x_k", bufs=4)
                xs = xkT[:, 0:QCS].rearrange("(t p) s -> p t s", p=128)
                for i in range(DT):
                    nc.sync.dma_start(wk_sb[i][:], wkT[i * 128:(i + 1) * 128, :])
                    nc.sync.dma_start(x_t[:, i:i + 1, :], xs[:, i:i + 1, :])
                xk_tiles[0] = x_t

            def kproj_group(kc, dt):
                """Project K chunk kc (k tiles 4kc..) for head-dim tile dt."""
                x_t = xk_tiles[kc]
                ps = fil_tile("ps_k")
                for i in range(DT):
                    nc.tensor.matmul(
                        ps[:], wk_sb[i][:, dt * 128:(dt + 1) * 128],
                        x_t[:, i, :], start=(i == 0), stop=(i == DT - 1))
                nc.vector.tensor_scalar(
                    kt_sb[dt][:, kc * QCS:(kc + 1) * QCS], ps[:],
                    1.0, bk_sb[:, dt:dt + 1],
                    mybir.AluOpType.mult, mybir.AluOpType.add)

            load_xk0_interleaved()
            kproj_group(0, 0)

            # wq + xq0 next (Q group (0, dt0) gates the attention start)
            for i in range(DT):
                nc.sync.dma_start(wq_sb[i][:], wqT[i * 128:(i + 1) * 128, :])

            xv_tiles = {}

            def load_xv(st4, eng):
                xv4 = xpool.tile([128, DT, QCS], BF16, tag="xv", name="xv4",
                                 bufs=2)
                eng.dma_start(
                    xv4[:],
                    xvT[:, st4 * QCS:(st4 + 1) * QCS].rearrange(
                        "(t p) s -> p t s", p=128))
                xv_tiles[st4] = xv4

            qt_tiles = {}
            xq_tiles = {}

            def load_xq(qc):
                qt_tiles[qc] = pp.tile([128, DH // 128, QCS], BF16, tag="qt",
                                       name="qt_t", bufs=2)
                xq2 = [wp.tile([128, DT // 2, QCS], BF16, tag="xq", name="xq2",
                               bufs=2) for _ in range(2)]
                for c in range(2):
                    xs = xqT[c * (D // 2):(c + 1) * (D // 2),
                             qc * QCS:(qc + 1) * QCS].rearrange(
                        "(t p) s -> p t s", p=128)
                    nc.sync.dma_start(xq2[c][:], xs)
                xq_tiles[qc] = xq2

            def qproj_pair(qc, dt0, ndt=2):
                """Q projection for chunk qc, head-dim tiles dt0..dt0+ndt."""
                qt_t = qt_tiles[qc]
                xq2 = xq_tiles[qc]
                for s in range(ndt):
                    dt = dt0 + s
                    ps = fil_tile("ps_qp")
                    for i in range(DT):
                        nc.tensor.matmul(
                            ps[:], wq_sb[i][:, dt * 128:(dt + 1) * 128],
                            xq2[i // (DT // 2)][:, i % (DT // 2), :],
                            start=(i == 0), stop=(i == DT - 1))
                    nc.vector.tensor_scalar(
                        qt_t[:, dt, :], ps[:], 0.125, bq_sb[:, dt:dt + 1],
                        mybir.AluOpType.mult, mybir.AluOpType.add)

            # ---- Q projection (0, dt0) in the head; dt1-3 ride as fillers.
            # The rest of the qc0 traffic (wv, xv0, xk1) follows on the same
            # queues; later pieces are issued from inside the attention loop
            # so they cannot jump ahead of the critical path on the global
            # DMA device.
            load_xq(0)
            qproj_pair(0, 0, ndt=1)
            for i in range(DT):
                nc.sync.dma_start(wv_sb[i][:], wvT[i * 128:(i + 1) * 128, :])
            load_xv(0, nc.sync)
            load_xk(1, nc.gpsimd)

            def vproj_pair(kt):
                """Project V for k tiles kt, kt+1 (two filler psum groups)."""
                if kt == 8:
                    load_xv(2, nc.sync)
                    load_xv(3, nc.gpsimd)
                for sub in range(2):
                    st = kt + sub
                    xv4 = xv_tiles[st // 4]
                    o = st % 4
                    ps = fil_tile("ps_v")
                    for i in range(DT):
                        nc.tensor.matmul(
                            ps[:], xv4[:, i, o * 128:(o + 1) * 128],
                            wv_sb[i][:], start=(i == 0), stop=(i == DT - 1))
                    vdst = v_sb[st].rearrange("p (h c) -> p h c", h=HPG)[:, :, 1:65]
                    nc.vector.tensor_copy(
                        vdst, ps[:].rearrange("p (h c) -> p h c", h=HPG))

            def emit_outproj_pair(cpair_sb, qc, ots, tail=False):
                """Output projection for two 128-row blocks (filler groups)."""
                for ot in ots:
                    po = fil_tile("po")
                    for j in range(HPG // 2):
                        nc.tensor.matmul(
                            po[:], wo_sb[j][:, ot * 128:(ot + 1) * 128],
                            cpair_sb[:, j, :],
                            start=(j == 0), stop=(j == HPG // 2 - 1))
                    o_sb = wp.tile([128, QCS], F32, tag="o", name="o_sb", bufs=3)
                    if tail and ot % 2 == 0:
                        nc.scalar.activation(
                            o_sb[:], po[:],
                            mybir.ActivationFunctionType.Identity,
                            bias=vr_sb[:, ot:ot + 1])
                    else:
                        nc.vector.tensor_scalar(
                            o_sb[:], po[:], 1.0, vr_sb[:, ot:ot + 1],
                            mybir.AluOpType.mult, mybir.AluOpType.add)
                    eng = nc.gpsimd if ot % 2 else nc.sync
                    eng.dma_start(
                        outT[ot * 128:(ot + 1) * 128, qc * QCS:(qc + 1) * QCS],
                        o_sb[:])

            def load_mask(qc, split=False):
                mask_sb = wp.tile([128, KT, QCS], BF16, tag="mask", name="mask_sb",
                                  bufs=2)
                ms = maskT[:, qc * QCS:(qc + 1) * QCS].rearrange(
                    "(t p) s -> p t s", p=128)
                if split:
                    # only the first half here: the kt8-15 pieces are issued
                    # from inside (qc0, j0) so they cannot jump ahead of the
                    # wq/xq0 critical path on the global DMA device
                    nc.sync.dma_start(mask_sb[:, 0:4, :], ms[:, 0:4, :])
                    nc.sync.dma_start(mask_sb[:, 4:8, :], ms[:, 4:8, :])
                else:
                    hm = KT // 2
                    nc.sync.dma_start(mask_sb[:, 0:hm, :], ms[:, 0:hm, :])
                    nc.gpsimd.dma_start(mask_sb[:, hm:KT, :], ms[:, hm:KT, :])
                return mask_sb

            def make_tp(nrm, cpair_t, j):
                """Deferred: transpose pair j's normalized C' into cpair."""
                def emit():
                    # transpose C'[q, (hh, d)] -> cpair[(hh, d), (qsub, q)]:
                    # one full-128-partition transpose per q-subtile (the
                    # pending-zero marking is partition-ranged, so partial-
                    # partition writes would land on unzeroed PSUM)
                    tp = fil_tile("tp")
                    tpb = tp.bitcast(BF16)  # [128, 1024] bf16 view
                    for qsub in range(4):
                        nc.tensor.matmul(
                            tpb[:, qsub * 128:(qsub + 1) * 128],
                            nrm[:, qsub, :, :], ident[:],
                            is_transpose=True,
                            start=(qsub == 0), stop=True,
                            skip_group_check=True)
                    nc.vector.tensor_copy(cpair_t[:, j, :], tpb[:, 0:QCS])
                return emit

            def emit_pv(cacc, j, kt, e_t):
                for qsub in range(4):
                    for hh in range(2):
                        b, g = qsub // 2, (qsub % 2) * 2 + hh
                        h = 2 * j + hh
                        nc.tensor.matmul(
                            cacc[b][:, g * 65:(g + 1) * 65],
                            e_t[:, hh, qsub * 128:(qsub + 1) * 128],
                            v_sb[kt][:, h * 65:(h + 1) * 65],
                            start=(kt == 0 and g == 0),
                            stop=(kt == KT - 1),
                            skip_group_check=True)

            # In (qc0, j0) the V projection and P@V are deferred a few k
            # tiles behind the scores so the in-order PE queue never parks
            # on the (DMA-bound) wv/xv/mask arrivals while the scalar
            # engine still has score tiles to chew through.
            VDEF, PVDEF = 6, 10

            def score_unit(qc, j, kt, qt_cur, mask_sb):
                """Scores + exp + mask for one (pair, k-tile); returns e."""
                e_sb = wp.tile([128, 2, QCS], BF16, tag="e", name="e_sb",
                               bufs=9)
                if (qc > 0 or j > 0) and kt % 4 == 2:
                    # offload 1/4 of the exp to the (otherwise idle)
                    # gpsimd engine: Schraudolph fast exp --
                    # e = bitcast_f32(int32(x * 2^23/ln2 + B)), then the
                    # mask multiply is fused into the bitcast read.
                    # ~1.8% elementwise rms; ~1% on the output. These
                    # pairs' scores use the psF banks so the sps rotation
                    # serves only scalar-engine units (keeps the exp
                    # pipeline 2 ACT-units deep).
                    so = [fil_tile("so0"), fil_tile("so1")]
                    for hh in range(2):
                        prow = hh * 64
                        nc.tensor.matmul(
                            so[hh][:],
                            kt_sb[j][prow:prow + 64,
                                     kt * 128:(kt + 1) * 128],
                            qt_cur[prow:prow + 64, j, :],
                            start=True, stop=True)
                    # gpsimd cannot read PSUM on HW: the int32 convert runs
                    # on the DVE; the bitcast mask-multiplies run on gpsimd
                    ei = wp.tile([128, 2, QCS], mybir.dt.int32,
                                 tag="ei", name="ei", bufs=2)
                    ef = ei.bitcast(F32)
                    for hh in range(2):
                        nc.vector.tensor_scalar(
                            ei[:, hh, :], so[hh][:], FEXP_A, FEXP_B,
                            mybir.AluOpType.mult, mybir.AluOpType.add)
                    for hh in range(2):
                        nc.gpsimd.tensor_mul(
                            e_sb[:, hh, :], ef[:, hh, :], mask_sb[:, kt, :])
                else:
                    sps = sps_tile("sps")
                    for hh in range(2):
                        prow = hh * 64
                        nc.tensor.matmul(
                            sps[:, hh, :],
                            kt_sb[j][prow:prow + 64,
                                     kt * 128:(kt + 1) * 128],
                            qt_cur[prow:prow + 64, j, :],
                            start=True, stop=True)
                    nc.scalar.activation(
                        e_sb[:], sps[:], mybir.ActivationFunctionType.Exp)
                    # steady state: over half the masks ride gpsimd -- the
                    # DVE carries the deferred normalize at the boundary
                    # plus the fast-exp int32 converts
                    meng = (nc.gpsimd
                            if (qc == 0 and j == 0 and kt % 4 == 3) or
                               ((qc > 0 or j > 0) and (kt < 4 or kt % 3 == 1))
                            else nc.vector)
                    for hh in range(2):
                        meng.tensor_mul(
                            e_sb[:, hh, :], e_sb[:, hh, :], mask_sb[:, kt, :])
                return e_sb

            def make_fin(cacc, j, e_list, cpair_t):
                """Deferred pair finish: trailing P@V groups + normalize.
                Runs inside the NEXT pair's first unit (after its scores and
                exp are already in flight) so the boundary never idles the
                scalar engine. Sets tp_holder['fn'] for the next pair's
                transpose filler."""
                def fin():
                    for pkt, pe in e_list:
                        emit_pv(cacc, j, pkt, pe)
                    # normalize: C'[q, (z, d)] per (qsub, hh); z = column
                    # g*65 (a per-partition scalar -- no broadcast needed).
                    zs = wp.tile([128, 2, 8], F32, tag="zs", name="zs", bufs=2)
                    for b in range(2):
                        nc.vector.tensor_copy(
                            zs[:, 0, 4 * b:4 * (b + 1)],
                            cacc[b][:, 0:260].rearrange(
                                "p (g c) -> p g c", c=65)[:, :, 0])
                    nc.vector.reciprocal_approx_fast(
                        out=zs[:, 1, :], in_=zs[:, 0, :])
                    nrm = wp.tile([128, 4, 2, DK], BF16, tag="nrm", name="nrm",
                                  bufs=2)
                    for qsub in range(4):
                        for hh in range(2):
                            b, g = qsub // 2, (qsub % 2) * 2 + hh
                            nc.vector.tensor_scalar_mul(
                                nrm[:, qsub, hh, :],
                                cacc[b][:, g * 65 + 1:(g + 1) * 65],
                                zs[:, 1, 4 * b + g:4 * b + g + 1])
                    tp_holder['fn'] = make_tp(nrm, cpair_t, j)
                return fin

            mask_next = load_mask(0, split=True)
            prev = None        # (cpair_sb, qc) pending output projection
            pending_fin = None  # deferred finish for the prior pair
            hoist_store = {}    # j -> score units emitted ahead of its loop
            tp_holder = {'fn': None}

            def pop_tp():
                fn, tp_holder['fn'] = tp_holder['fn'], None
                if fn is not None:
                    fn()

            for qc in range(QCN):
                mask_sb = mask_next
                qt_cur = qt_tiles[qc]
                cpair_t = wp.tile([128, HPG // 2, QCS], BF16, tag="cp",
                                  name="cpair_t", bufs=2)
                for j in range(HPG // 2):
                    dtile = j
                    # filler closures, emitted one-per-unit inside the kt
                    # loop so the in-order PE queue never parks a big block.
                    # In qc0 the K projection is spread dt-major: pair j
                    # carries K(chunk 1-3, dt=j) for its own k tiles plus
                    # K(chunk 0, dt=j+1) and Q(0, dt=j+1) for the next pair.
                    fillers = []
                    if qc == 0:
                        fillers.append(lambda d=j: kproj_group(1, d))
                        fillers.append(pop_tp)
                        if j < 3:
                            fillers.append(
                                lambda d=j + 1: qproj_pair(0, d, ndt=1))
                            fillers.append(lambda d=j + 1: kproj_group(0, d))
                        fillers.append(lambda d=j: kproj_group(2, d))
                        fillers.append(lambda d=j: kproj_group(3, d))
                    else:
                        if prev is not None:
                            for ot in (2 * j, 2 * j + 1):
                                fillers.append(
                                    lambda o=ot, pv=prev: emit_outproj_pair(
                                        pv[0], pv[1], [o]))
                        # at j0 the held transpose is the previous chunk's
                        # j3 (finished inline): it must land before the po
                        # fillers that read the completed cpair
                        fillers.insert(0 if j == 0 else min(2, len(fillers)),
                                       pop_tp)
                    if qc + 1 < QCN and j >= 2:
                        for s in range(2):
                            fillers.append(
                                lambda d=2 * (j - 2) + s, q=qc + 1:
                                qproj_pair(q, d, ndt=1))
                    # C' accumulators: bank b holds q-subtiles 2b, 2b+1;
                    # four 65-col groups per bank (g = (qsub%2)*2 + hh).
                    cacc = [cacc_tile(f"cacc{b}") for b in range(2)]
                    defer = qc == 0 and j == 0
                    # P@V trails the exp/mask producers so the in-order PE
                    # queue never parks on them: 8 units behind in (qc0, j0)
                    # (DMA-bound V/mask arrivals), 2 units elsewhere (Pool
                    # fast-exp latency).
                    pv_lag = PVDEF if defer else 6
                    e_pend = []  # (kt, e_t) awaiting deferred P@V
                    hoisted = hoist_store.pop(j, [])
                    for kt in range(KT):
                        if kt < len(hoisted):
                            # score unit already emitted inside the previous
                            # pair's V / P@V tail
                            e_pend.append((kt, hoisted[kt]))
                            if kt == 0 and pending_fin is not None:
                                pending_fin()
                                pending_fin = None
                            if fillers:
                                fillers.pop(0)()
                            continue
                        if defer:
                            # late DMA issues, ordered behind the critical
                            # path on their queues
                            if kt == 1:
                                load_xk(2, nc.sync)
                                ms0 = maskT[:, 0:QCS].rearrange(
                                    "(t p) s -> p t s", p=128)
                                nc.gpsimd.dma_start(
                                    mask_sb[:, 8:12, :], ms0[:, 8:12, :])
                                nc.gpsimd.dma_start(
                                    mask_sb[:, 12:16, :], ms0[:, 12:16, :])
                            elif kt == 2:
                                load_xv(1, nc.gpsimd)
                            elif kt == 3:
                                load_xk(3, nc.sync)
                            elif kt == 6:
                                for i in range(DH // 128):
                                    nc.gpsimd.dma_start(
                                        wo_sb[i][:],
                                        woT[i * 128:(i + 1) * 128, :])
                        e_sb = score_unit(qc, j, kt, qt_cur, mask_sb)
                        e_pend.append((kt, e_sb))
                        if kt == 0 and pending_fin is not None:
                            pending_fin()
                            pending_fin = None
                        if defer and kt >= VDEF and (kt - VDEF) % 2 == 0:
                            vproj_pair(kt - VDEF)
                        if kt >= pv_lag:
                            pkt, pe = e_pend.pop(0)
                            emit_pv(cacc, j, pkt, pe)
                        # one filler per unit keeps every PE lump <= ~1.7us
                        # (qc0 pops every unit -- it has more to place; the
                        # steady state pops at odd kt so the pair-boundary
                        # normalize/transpose chain gets a head start)
                        if fillers and (qc == 0 or kt % 2 == 1):
                            fillers.pop(0)()
                    if defer:
                        # V / P@V tail of (qc0, j0), interleaved with pair
                        # j1's first score units so the scalar engine never
                        # drains while the PE finishes the projections.
                        hoist = hoist_store.setdefault(1, [])
                        for st in (10, 12, 14):
                            vproj_pair(st)
                            hoist.append(
                                score_unit(qc, 1, len(hoist), qt_cur, mask_sb))
                            for _ in range(2):
                                if e_pend:
                                    pkt, pe = e_pend.pop(0)
                                    emit_pv(cacc, j, pkt, pe)
                    # j3's finish runs inline: its transpose gates the next
                    # chunk's output projections, so it cannot wait for a
                    # mid-loop filler slot there.
                    pending_fin = make_fin(cacc, j, list(e_pend), cpair_t)
                    if j == HPG // 2 - 1:
                        pending_fin()
                        pending_fin = None
                    if j == 0 and qc + 1 < QCN:
                        mask_next = load_mask(qc + 1)
                        load_xq(qc + 1)
                prev = (cpair_t, qc)
            # drain: last pair's transpose, then the last output projection
            pop_tp()
            for p in range(4):
                emit_outproj_pair(prev[0], prev[1], range(2 * p, 2 * p + 2),
                                  tail=True)

    nc.finalize()
    return nc


_NC_CACHE = None


def _get_nc():
    global _NC_CACHE
    if _NC_CACHE is None:
        _NC_CACHE = build_nc()
    return _NC_CACHE


def shard_inputs(query, key, value, mask, wq, bq, wk, bk, wv, bv, wo, bo):
    """Build the per-core input maps (host-side shard prep)."""
    import ml_dtypes

    query = np.asarray(query, np.float32)
    key = np.asarray(key, np.float32)
    value = np.asarray(value, np.float32)
    mask = np.asarray(mask)
    wq = np.asarray(wq, np.float32); bq = np.asarray(bq, np.float32)
    wk = np.asarray(wk, np.float32); bk = np.asarray(bk, np.float32)
    wv = np.asarray(wv, np.float32); bv = np.asarray(bv, np.float32)
    wo = np.asarray(wo, np.float32); bo = np.asarray(bo, np.float32)

    in_maps = []
    bf = ml_dtypes.bfloat16
    maskT_b = [np.ascontiguousarray(mask[b].T).astype(bf) for b in range(B)]
    xT = {}
    for b in range(B):
        xT[b] = (
            np.ascontiguousarray(query[b].T.astype(bf)),
            np.ascontiguousarray(key[b].T.astype(bf)),
            np.ascontiguousarray(value[b].T.astype(bf)),
        )
    for c in range(NCORES):
        b, hg = divmod(c, HG)
        sl = slice(hg * DH, (hg + 1) * DH)
        wo_block = wo[:, sl]                       # [1024, 512]
        v_r = bv[sl] @ wo_block.T                  # [1024]
        if hg == 0:
            v_r = v_r + bo
        in_maps.append({
            "xqT": xT[b][0],
            "xkT": xT[b][1],
            "xvT": xT[b][2],
            "maskT": maskT_b[b],
            "wqT": np.ascontiguousarray(wq[sl].T.astype(bf)),
            "wkT": np.ascontiguousarray(wk[sl].T.astype(bf)),
            "wvT": np.ascontiguousarray(wv[sl].T.astype(bf)),
            "woT": np.ascontiguousarray(wo_block.T.astype(bf)),
            "bq2": np.ascontiguousarray((bq[sl] / 8.0).reshape(DH // 128, 128).T),
            "bk2": np.ascontiguousarray(bk[sl].reshape(DH // 128, 128).T),
            "vr2": np.ascontiguousarray(v_r.reshape(D // 128, 128).T),
        })
    return in_maps


def combine_outputs(results):
    """results: list of per-core {"outT": [1024, 2048]} -> full [B, S, D]."""
    out = np.empty((B, S, D), np.float32)
    for b in range(B):
        acc = results[2 * b]["outT"] + results[2 * b + 1]["outT"]
        out[b] = acc.T
    return out


def kernel(**inputs):
    from concourse.bass_utils import run_bass_kernel_spmd

    nc = _get_nc()
    in_maps = shard_inputs(**inputs)
    res = run_bass_kernel_spmd(nc, in_maps, list(range(NCORES)))
    return combine_outputs(res.results)
